# revision 1
# baseline (speedup 1.0000x reference)
"""Trainium2 Bass kernel for nn_Block_47098611368060 (dense transformer block).

Sharding: 8 cores = 4 batches x 2 parity groups. Core (b, p) owns the
interleaved query blocks {2j+p : j=0..7} (128 rows each) of batch b and
computes them end-to-end: LN1 -> QKV -> causal attention -> proj ->
residual -> LN2 -> MLP(gelu-tanh) -> residual.  K/V are computed locally
for the full 2048-row sequence, so no collectives / cross-core traffic
at all.  Causal structure is handled with a per-core additive tail mask
(identical program on all cores; only data differs).
"""

import sys

for _p in ("/opt/trn_rl_repo",):
    if _p not in sys.path:
        sys.path.insert(0, _p)

import math
import numpy as np

import concourse.bass as bass
import concourse.tile as tile
from concourse import bacc, mybir
from concourse.masks import make_identity

F32 = mybir.dt.float32
F32R = mybir.dt.float32r

P = 128          # partitions
EPS = 1e-6
NEG = -1e9


class Cfg:
    def __init__(self, S=2048, D=1024, NH=16, HD=64, HID=4096, NC=512):
        self.S, self.D, self.NH, self.HD, self.HID = S, D, NH, HD, HID
        self.NC = NC                  # moving-operand chunk (<= 512 for fp32)
        self.SQ = S // 2              # own query rows per core
        self.RB = S // P              # seq row blocks
        self.QB = self.SQ // P        # own query blocks
        self.DB = D // P              # model-dim feature blocks
        self.HB = HID // P            # hidden feature blocks
        assert D % P == 0 and S % (2 * P) == 0 and HID % P == 0
        assert NH * HD == D and HD <= P
        assert NC >= 2 * P and self.SQ % NC == 0 and D % NC == 0 and S % NC == 0
        assert self.QB % 2 == 0


def _bcast(ap, parts, n):
    """[n] dram AP -> [parts, n] partition-broadcast AP."""
    return bass.AP(tensor=ap.tensor, offset=ap.offset, ap=[[0, parts]] + list(ap.ap))


def build(nc, tc, cfg, use_f32r=False, reps=1, stop_after=None):
    """Emit the full per-core program. reps>1 wraps the whole body in a
    device-side loop (benchmark amplification only). use_f32r runs every
    matmul in fp32r (full-rate PE; ~1.5e-4 operand rounding)."""
    import contextlib
    c = cfg
    NC = c.NC
    scale = 1.0 / math.sqrt(c.HD)
    DT = F32R if use_f32r else F32   # matmul-operand dtype

    def mm(out, lhsT, rhs, start, stop, dt=None):
        nc.tensor.matmul(out, lhsT, rhs, start=start, stop=stop)

    # ---- I/O ----
    x_full = nc.dram_tensor("x_full", [c.S, c.D], F32, kind="ExternalInput").ap()
    x_own = nc.dram_tensor("x_own", [c.SQ, c.D], F32, kind="ExternalInput").ap()
    maskd = nc.dram_tensor("mask", [P, 4, 2 * P], F32, kind="ExternalInput").ap()
    w_qkv = nc.dram_tensor("w_qkv", [c.D, 3 * c.D], DT, kind="ExternalInput").ap()
    w_proj = nc.dram_tensor("w_proj", [c.D, c.D], DT, kind="ExternalInput").ap()
    w1 = nc.dram_tensor("w1", [c.D, c.HID], DT, kind="ExternalInput").ap()
    w2 = nc.dram_tensor("w2", [c.HID, c.D], DT, kind="ExternalInput").ap()
    ln1_s = nc.dram_tensor("ln1_scale", [c.D], F32, kind="ExternalInput").ap()
    ln1_b = nc.dram_tensor("ln1_bias", [c.D], F32, kind="ExternalInput").ap()
    ln2_s = nc.dram_tensor("ln2_scale", [c.D], F32, kind="ExternalInput").ap()
    ln2_b = nc.dram_tensor("ln2_bias", [c.D], F32, kind="ExternalInput").ap()
    b_proj = nc.dram_tensor("b_proj", [c.D], F32, kind="ExternalInput").ap()
    b1 = nc.dram_tensor("b1", [c.HID], F32, kind="ExternalInput").ap()
    b2 = nc.dram_tensor("b2", [c.D], F32, kind="ExternalInput").ap()
    out = nc.dram_tensor("out", [c.SQ, c.D], F32, kind="ExternalOutput").ap()

    # ---- DRAM scratch ----
    qT_s = nc.dram_tensor("qT_s", [c.D, c.SQ], DT).ap()
    kT_s = nc.dram_tensor("kT_s", [c.D, c.S], DT).ap()
    v_s = nc.dram_tensor("v_s", [c.S, c.D], DT).ap()
    x2_s = nc.dram_tensor("x2_s", [c.SQ, c.D], F32).ap()

    BN_FMAX = nc.vector.BN_STATS_FMAX
    BN_SD = nc.vector.BN_STATS_DIM
    BN_AD = nc.vector.BN_AGGR_DIM

    rep_loop = tc.For_i(0, reps, 1) if reps > 1 else contextlib.nullcontext()
    with rep_loop, tc.tile_pool(name="singles", bufs=1) as singles:
        ident = singles.tile([P, P], F32)
        make_identity(nc, ident)
        eps_t = singles.tile([P, 1], F32)
        nc.vector.memset(eps_t, EPS)
        mask_sb = singles.tile([P, 4, 2 * P], F32)
        nc.sync.dma_start(mask_sb, maskd)

        ln1_sc = singles.tile([P, c.D], F32)
        nc.sync.dma_start(ln1_sc, _bcast(ln1_s, P, c.D))
        ln1_bi = singles.tile([P, c.D], F32)
        nc.sync.dma_start(ln1_bi, _bcast(ln1_b, P, c.D))
        ln2_sc = singles.tile([P, c.D], F32)
        nc.sync.dma_start(ln2_sc, _bcast(ln2_s, P, c.D))
        ln2_bi = singles.tile([P, c.D], F32)
        nc.sync.dma_start(ln2_bi, _bcast(ln2_b, P, c.D))
        bproj_b = singles.tile([P, c.D], F32)
        nc.sync.dma_start(bproj_b, _bcast(b_proj, P, c.D))
        b2_b = singles.tile([P, c.D], F32)
        nc.sync.dma_start(b2_b, _bcast(b2, P, c.D))
        b1_sb = singles.tile([P, c.HB], F32)
        nc.sync.dma_start(b1_sb, b1.rearrange("(o p) -> p o", p=P))

        def layernorm(pool, x_t, sc_t, bi_t, y_t):
            """Row-major LN: y = (x - mu) * rsqrt(var+eps) * scale + bias."""
            sub = math.gcd(BN_FMAX, c.D)
            nsub = c.D // sub
            xg = x_t.rearrange("p (n s) -> p n s", s=sub)
            st = pool.tile([P, nsub, BN_SD], F32, tag="ln_st")
            for i in range(nsub):
                nc.vector.bn_stats(st[:, i, :], xg[:, i, :])
            mv = pool.tile([P, BN_AD], F32, tag="ln_mv")
            nc.vector.bn_aggr(mv, st)
            std = pool.tile([P, 1], F32, tag="ln_std")
            nc.scalar.activation(std, mv[:, 1:2],
                                 mybir.ActivationFunctionType.Sqrt,
                                 bias=eps_t, scale=1.0)
            rstd = pool.tile([P, 1], F32, tag="ln_rstd")
            nc.vector.reciprocal(rstd, std)
            nc.vector.tensor_scalar(y_t, x_t, mv[:, 0:1], rstd,
                                    op0=mybir.AluOpType.subtract,
                                    op1=mybir.AluOpType.mult)
            nc.vector.tensor_mul(y_t, y_t, sc_t)
            nc.vector.tensor_add(y_t, y_t, bi_t)

        out_b4 = out.rearrange("(rb p) (f q) -> rb p f q", p=P, q=P)

        def dump_and_stop(src3):  # src3: [P, DB, >=SQ] sbuf tile
            for rb in range(c.QB):
                nc.sync.dma_start(out_b4[rb],
                                  src3[:, :, rb * P:(rb + 1) * P].bitcast(F32))

        # ============ Phase A: LN1 + transpose ============
        with tc.tile_pool(name="yT_pool", bufs=1) as yT_pool:
            yT = yT_pool.tile([P, c.DB, c.S], DT)
            yTo = yT_pool.tile([P, c.DB, c.SQ], DT)
            with tc.tile_pool(name="ln_work", bufs=3) as lnw, \
                 tc.tile_pool(name="tp_ps", bufs=4, space="PSUM") as tp_ps:

                def ln_transpose(src_blocked, nblocks, dst):
                    for rb in range(nblocks):
                        x_t = lnw.tile([P, c.D], F32, tag="ln_x")
                        nc.sync.dma_start(x_t, src_blocked[rb])
                        y_t = lnw.tile([P, c.D], F32, tag="ln_y")
                        layernorm(lnw, x_t, ln1_sc, ln1_bi, y_t)
                        for f in range(c.DB):
                            pt = tp_ps.tile([P, P], F32, tag="tp")
                            nc.tensor.transpose(
                                pt, y_t[:, f * P:(f + 1) * P], ident)
                            nc.vector.tensor_copy(
                                dst[:, f, rb * P:(rb + 1) * P], pt)

                ln_transpose(x_full.rearrange("(rb p) d -> rb p d", p=P), c.RB, yT)
                ln_transpose(x_own.rearrange("(rb p) d -> rb p d", p=P), c.QB, yTo)
            if stop_after == "A":
                dump_and_stop(yT)
                return

            # ============ Phase B: QKV -> DRAM scratch ============
            with tc.tile_pool(name="qkv_w", bufs=2) as wp, \
                 tc.tile_pool(name="qkv_ps", bufs=3, space="PSUM") as qps, \
                 tc.tile_pool(name="qkv_st", bufs=4) as stp:
                for (n_rows, src, dst, col0, do_scale) in (
                        (c.SQ, yTo, qT_s, 0, True),
                        (c.S, yT, kT_s, c.D, False)):
                    for fo in range(c.DB):
                        wt = wp.tile([P, c.DB, P], DT, tag="w_qk")
                        wcol = w_qkv[:, col0 + fo * P: col0 + (fo + 1) * P]
                        nc.sync.dma_start(
                            wt, wcol.rearrange("(o p) q -> p o q", p=P))
                        for ch in range(n_rows // NC):
                            ps = qps.tile([P, NC], F32, tag="qk_ps")
                            for f in range(c.DB):
                                mm(ps, wt[:, f, :],
                                   src[:, f, ch * NC:(ch + 1) * NC],
                                   start=(f == 0), stop=(f == c.DB - 1))
                            st = stp.tile([P, NC], DT, tag="qk_st")
                            if do_scale:
                                nc.scalar.mul(st, ps, scale)
                            else:
                                nc.scalar.copy(st, ps)
                            nc.sync.dma_start(
                                dst[fo * P:(fo + 1) * P, ch * NC:(ch + 1) * NC],
                                st)
                for vc in range(c.D // NC):
                    wv = wp.tile([P, c.DB, NC], DT, tag="w_v")
                    wcol = w_qkv[:, 2 * c.D + vc * NC: 2 * c.D + (vc + 1) * NC]
                    nc.sync.dma_start(wv, wcol.rearrange("(o p) q -> p o q", p=P))
                    for rb in range(c.RB):
                        ps = qps.tile([P, NC], F32, tag="v_ps")
                        for f in range(c.DB):
                            mm(ps, yT[:, f, rb * P:(rb + 1) * P], wv[:, f, :],
                               start=(f == 0), stop=(f == c.DB - 1))
                        st = stp.tile([P, NC], DT, tag="v_st")
                        nc.scalar.copy(st, ps)
                        nc.sync.dma_start(
                            v_s[rb * P:(rb + 1) * P, vc * NC:(vc + 1) * NC], st)
            if stop_after == "B":
                dump_and_stop(yT)
                return

        # ===== Phase C: attention v2 (St = K@Q^T; denominator via V|1) =====
        KMAX = 2 * c.QB * P  # == S
        with tc.tile_pool(name="OT_pool", bufs=1) as OTp:
            OT = OTp.tile([P, c.DB, c.SQ], DT)
            ones_rb = OTp.tile([P, c.RB, 1], F32)
            nc.vector.memset(ones_rb, 1.0)
            with tc.tile_pool(name="at_in", bufs=3) as aip, \
                 tc.tile_pool(name="at_e", bufs=2) as ep, \
                 tc.tile_pool(name="at_sm", bufs=8) as smp, \
                 tc.tile_pool(name="at_sps", bufs=4, space="PSUM") as spsp, \
                 tc.tile_pool(name="at_ops", bufs=2, space="PSUM") as opsp:
                for h in range(c.NH):
                    qTh = aip.tile([c.HD, c.SQ], DT, tag="qTh")
                    nc.sync.dma_start(qTh, qT_s[h * c.HD:(h + 1) * c.HD, :])
                    kTh = aip.tile([c.HD, c.S], DT, tag="kTh")
                    nc.sync.dma_start(kTh, kT_s[h * c.HD:(h + 1) * c.HD, :])
                    vh = aip.tile([P, c.RB, c.HD + 1], DT, tag="vh")
                    nc.sync.dma_start(
                        vh[:, :, :c.HD],
                        v_s[:, h * c.HD:(h + 1) * c.HD]
                        .rearrange("(rb p) d -> p rb d", p=P))
                    nc.vector.tensor_copy(vh[:, :, c.HD:], ones_rb)
                    fo, fi = h // 2, (h % 2) * c.HD  # OT feature placement
                    for t in range(c.QB // 2):
                        j0, j1 = 2 * t, 2 * t + 1
                        nkb0 = 2 * j0 + 2
                        nkb1 = 2 * j1 + 2
                        E = ep.tile([P, nkb1, 2 * P], DT, tag="E",
                                    name=f"E_{t}")
                        ops = opsp.tile([c.HD + 1, 2, P], F32, tag="o_ps")
                        opsf = ops.rearrange("d a b -> d (a b)")
                        for kb in range(nkb1):
                            st = spsp.tile([P, 2 * P], F32, tag="st_ps")
                            # St[k, (a q)] for the query pair
                            nc.tensor.matmul(
                                st, kTh[:, kb * P:(kb + 1) * P],
                                qTh[:, j0 * P: j0 * P + 2 * P],
                                start=True, stop=True)
                            mi = kb - (nkb0 - 2)
                            if 0 <= mi < 4:
                                nc.vector.tensor_add(st, st, mask_sb[:, mi, :])
                            nc.scalar.activation(
                                E[:, kb, :], st,
                                mybir.ActivationFunctionType.Exp)
                            nc.tensor.matmul(
                                opsf, vh[:, kb, :], E[:, kb, :],
                                start=(kb == 0), stop=(kb == nkb1 - 1))
                        for a, j in ((0, j0), (1, j1)):
                            rcp = smp.tile([1, P], F32, tag="rcp")
                            nc.vector.reciprocal(rcp, ops[c.HD:, a, :])
                            rb = smp.tile([c.HD, P], F32, tag="rb")
                            nc.gpsimd.partition_broadcast(rb, rcp)
                            nc.vector.tensor_mul(
                                OT[fi:fi + c.HD, fo, j * P:(j + 1) * P],
                                ops[:c.HD, a, :], rb)
            if stop_after == "C":
                dump_and_stop(OT)
                return

            # ====== Phase D1: proj + residual + LN2 + transpose ======
            with tc.tile_pool(name="y2T_pool", bufs=1) as y2Tp:
                y2T = y2Tp.tile([P, c.DB, c.SQ], DT)
                out_acc = y2Tp.tile([P, c.QB, c.D], F32)
                with tc.tile_pool(name="pr_w", bufs=1) as pwp, \
                     tc.tile_pool(name="pr_work", bufs=3) as prw, \
                     tc.tile_pool(name="pr_ps", bufs=3, space="PSUM") as prps, \
                     tc.tile_pool(name="pr_tps", bufs=3, space="PSUM") as prtps:
                    wproj_sb = pwp.tile([P, c.DB, c.D], DT)
                    nc.sync.dma_start(
                        wproj_sb, w_proj.rearrange("(o p) q -> p o q", p=P))
                    for rq in range(c.QB):
                        x2_t = prw.tile([P, c.D], F32, tag="x2")
                        for fc in range(c.D // NC):
                            ps = prps.tile([P, NC], F32, tag="pr_ps")
                            for hp in range(c.DB):
                                mm(ps, OT[:, hp, rq * P:(rq + 1) * P],
                                   wproj_sb[:, hp, fc * NC:(fc + 1) * NC],
                                   start=(hp == 0), stop=(hp == c.DB - 1))
                            xo = prw.tile([P, NC], F32, tag="xo")
                            nc.sync.dma_start(
                                xo, x_own[rq * P:(rq + 1) * P,
                                          fc * NC:(fc + 1) * NC])
                            sl = x2_t[:, fc * NC:(fc + 1) * NC]
                            nc.vector.tensor_add(sl, ps, xo)
                            nc.vector.tensor_add(
                                sl, sl, bproj_b[:, fc * NC:(fc + 1) * NC])
                        nc.vector.tensor_add(out_acc[:, rq, :], x2_t,
                                             b2_b)
                        y2_t = prw.tile([P, c.D], F32, tag="y2")
                        layernorm(prw, x2_t, ln2_sc, ln2_bi, y2_t)
                        for f in range(c.DB):
                            pt = prtps.tile([P, P], F32, tag="tp2")
                            nc.tensor.transpose(
                                pt, y2_t[:, f * P:(f + 1) * P], ident)
                            nc.vector.tensor_copy(
                                y2T[:, f, rq * P:(rq + 1) * P], pt)

                # ===== Phase D2: MLP (hidden-block streaming, SBUF accum) =====
                NRB = c.SQ // P
                NCH = c.SQ // NC
                with tc.tile_pool(name="mlp_w", bufs=3) as mwp, \
                     tc.tile_pool(name="mlp_h", bufs=3) as mhp, \
                     tc.tile_pool(name="mlp_gw", bufs=3) as mgw, \
                     tc.tile_pool(name="mlp_ps", bufs=3, space="PSUM") as mps, \
                     tc.tile_pool(name="m2_ps", bufs=4, space="PSUM") as m2ps:
                    for hb in range(c.HB):
                        w1t = mwp.tile([P, c.DB, P], DT, tag="w1t")
                        nc.sync.dma_start(
                            w1t, w1[:, hb * P:(hb + 1) * P]
                            .rearrange("(o p) q -> p o q", p=P))
                        w2row = mwp.tile([P, c.D], DT, tag="w2row")
                        nc.sync.dma_start(w2row, w2[hb * P:(hb + 1) * P, :])
                        h_hb = mhp.tile([P, NCH, NC], DT, tag="h_hb")
                        for chq in range(NCH):
                            ps = mps.tile([P, NC], F32, tag="h_ps")
                            for f in range(c.DB):
                                mm(ps, w1t[:, f, :],
                                   y2T[:, f, chq * NC:(chq + 1) * NC],
                                   start=(f == 0), stop=(f == c.DB - 1))
                            # gelu-tanh (host halves w2):
                            # x * (1 + tanh(0.79788456*(x + 0.044715 x^3)))
                            xg = mgw.tile([P, NC], F32, tag="g_x")
                            nc.scalar.activation(
                                xg, ps,
                                mybir.ActivationFunctionType.Identity,
                                bias=b1_sb[:, hb:hb + 1], scale=1.0)
                            u = mgw.tile([P, NC], F32, tag="g_u")
                            nc.vector.tensor_mul(u, xg, xg)
                            nc.vector.tensor_mul(u, u, xg)
                            nc.vector.scalar_tensor_tensor(
                                u, u, 0.044715, xg,
                                op0=mybir.AluOpType.mult,
                                op1=mybir.AluOpType.add)
                            nc.scalar.activation(
                                u, u, mybir.ActivationFunctionType.Tanh,
                                scale=0.7978845608028654)
                            nc.vector.scalar_tensor_tensor(
                                h_hb[:, chq, :], u, 1.0, xg,
                                op0=mybir.AluOpType.add,
                                op1=mybir.AluOpType.mult)
                        for rb in range(NRB):
                            chq, rbl = divmod(rb, NC // P)
                            for fc in range(c.D // NC):
                                ps2 = m2ps.tile([P, NC], F32, tag="m2_ps")
                                nc.tensor.matmul(
                                    ps2,
                                    h_hb[:, chq, rbl * P:(rbl + 1) * P],
                                    w2row[:, fc * NC:(fc + 1) * NC],
                                    start=True, stop=True)
                                sl = out_acc[:, rb, fc * NC:(fc + 1) * NC]
                                nc.vector.tensor_add(sl, sl, ps2)
                    ob3 = out.rearrange("(rb p) d -> rb p d", p=P)
                    for rb in range(NRB):
                        nc.sync.dma_start(ob3[rb], out_acc[:, rb, :])

# =================== host side ===================

def make_core_inputs(inputs, cfg, b, p):
    """Per-core input map for core (batch b, parity p)."""
    c = cfg
    x = np.asarray(inputs["x"], np.float32)
    xb = np.ascontiguousarray(x[b])              # [S, D]
    xob = xb.reshape(c.RB, P, c.D)[p::2]         # [QB, P, D]
    # transposed additive masks, keys on partitions: T[k,q]=0 iff k<=q
    T = np.where(np.arange(P)[:, None] <= np.arange(P)[None, :],
                 np.float32(0.0), np.float32(NEG)).astype(np.float32)
    F = np.full((P, P), NEG, np.float32)
    Z = np.zeros((P, P), np.float32)
    last2 = (T, F) if p == 0 else (Z, T)
    # maskC[:, i, :] added to St psum [P, 2*P] at the four causal-edge
    # key blocks: i0 -> kb=nkb0-2, i1 -> nkb0-1, i2 -> nkb0, i3 -> nkb0+1
    maskC = np.stack([
        np.concatenate([last2[0], Z], 1),
        np.concatenate([last2[1], Z], 1),
        np.concatenate([F, last2[0]], 1),
        np.concatenate([F, last2[1]], 1),
    ], axis=1)  # [P, 4, 2P]
    m = {
        "x_full": xb,
        "x_own": np.ascontiguousarray(xob.reshape(c.SQ, c.D)),
        "mask": np.ascontiguousarray(maskC.astype(np.float32)),
    }
    for k in ("w_qkv", "w_proj", "w1", "w2", "ln1_scale", "ln1_bias",
              "ln2_scale", "ln2_bias", "b_proj", "b1", "b2"):
        m[k] = np.ascontiguousarray(np.asarray(inputs[k], np.float32))
    # device emits gelu without the leading 0.5; fold it into w2
    m["w2"] = np.ascontiguousarray(m["w2"] * np.float32(0.5))
    return m


_CACHE = {}


def get_nc(cfg, use_f32r=False, enable_asserts=False, reps=1, stop_after=None):
    key = (cfg.S, cfg.D, cfg.NH, cfg.HID, cfg.NC, use_f32r, reps, stop_after)
    if key not in _CACHE:
        nc = bacc.Bacc("TRN2", target_bir_lowering=False, debug=False,
                       enable_asserts=enable_asserts, num_devices=8)
        with tile.TileContext(nc) as tc:
            build(nc, tc, cfg, use_f32r=use_f32r, reps=reps,
                  stop_after=stop_after)
        nc.compile()
        _CACHE[key] = nc
    return _CACHE[key]


USE_F32R = True


def kernel(**inputs):
    from concourse.bass_utils import run_bass_kernel_spmd
    cfg = Cfg()
    nc = get_nc(cfg, use_f32r=USE_F32R)
    in_maps = [make_core_inputs(inputs, cfg, i // 2, i % 2) for i in range(8)]
    res = run_bass_kernel_spmd(nc, in_maps, list(range(8))).results
    B = 4
    outf = np.empty((B, cfg.S, cfg.D), np.float32)
    ob = outf.reshape(B, cfg.RB, P, cfg.D)
    for i in range(8):
        b, p = i // 2, i % 2
        ob[b, p::2] = res[i]["out"].reshape(cfg.QB, P, cfg.D)
    return outf

